# revision 22
# baseline (speedup 1.0000x reference)
"""MoE routing kernel for Trainium2 (8 NeuronCores, batch-parallel), v4.

Per batch element b (one NeuronCore each):
    pooled = mean_s x[b]; h = tanh(pooled @ rw1 + rb1)
    logits = h @ rw2 + rb2; probs = softmax(logits)
    top-3 of 4 experts, renormalized; out[b] = x[b] + sum_e w[e] * z_e

v4 design (vs v3):
  - GEMM2 entirely fp8 DoubleRow (was half bf16). Afforded by scaling
    zw1 x32 and zw2 x64 host-side before fp8 cast (pulls sigma~1/32,
    1/64 weights out of the e4m3 subnormal range); compensated exactly
    by GELU scale=1/32 and combine-weights/64. No bf16 zw2 stream.
  - Router pools from the fp8 xT8 tiles (selection margin ~20x the fp8
    logit perturbation); the bf16 transposed x copy is gone.
  - Startup: rw1 + xt8 split across both rings and issued first;
    x (residual) deferred behind the router consts; first zw1 tile split
    across rings; scalar ring has its own register snaps and carries
    zb1[0] + the zw28 stream. Warm MMs bracket the router PE work.
  - gelu_warm takes a softmax-dependent bias so the scalar stream is
    tanh -> exp -> gelu (no GELU table reload mid-stream).
  - Each out chunk is written as two 128KB DMAs, one per ring.
"""
import sys

sys.path.insert(0, "/opt/trn_rl_repo")

import numpy as np
import ml_dtypes

import concourse.bacc as bacc
import concourse.bass as bass
import concourse.mybir as mybir
import concourse.tile as tile
from concourse.bass_utils import run_bass_kernel_spmd

S, D, F, E, H = 512, 1024, 4096, 4, 256
K = 3            # active experts (top-3 of 4)
P = 128
TC = S // P      # 4 token chunks
DC = D // P      # 8 d chunks
DP = DC // 2     # 4 d chunk-pairs (DoubleRow K=256)
FC = F // P      # 32 ff chunks
F32 = mybir.dt.float32
BF16 = mybir.dt.bfloat16
FP8 = mybir.dt.float8e4
I32 = mybir.dt.int32
GELU = mybir.ActivationFunctionType.Gelu_apprx_tanh
DR = mybir.MatmulPerfMode.DoubleRow
W1SCALE = 32.0   # zw1 pre-scale (host) / GELU scale compensation
W2SCALE = 64.0   # zw2 pre-scale (host) / combine-weight compensation
PRE1 = 13        # zw1 pair-tile prefetch depth (covers 2*PRE1 fc chunks)
NB28 = 4         # zw28 half-tile pool size (8KB/partition each)


def build_nc():
    nc = bacc.Bacc("TRN2", target_bir_lowering=False, debug=False)

    x_d = nc.dram_tensor("x", [S, D], BF16, kind="ExternalInput")
    xt8_d = nc.dram_tensor("xt8", [DP * P, 2, S], FP8, kind="ExternalInput")
    rw1_d = nc.dram_tensor("rw1b", [D, H], BF16, kind="ExternalInput")
    rb1_d = nc.dram_tensor("rb1", [H], F32, kind="ExternalInput")
    rw2_d = nc.dram_tensor("rw2", [H, E], F32, kind="ExternalInput")
    rb2_d = nc.dram_tensor("rb2", [E], F32, kind="ExternalInput")
    # zw1r8[e*P+p, fcp, j, dcp, i, fw] = 32*zw1[e, (2*dcp+i)*P+p, (2*fcp+j)*P+fw]
    zw1_d = nc.dram_tensor("zw1r", [E * P, FC // 2, 2, DP, 2, P], FP8,
                           kind="ExternalInput")
    # zb1r[e*P+p, fc] = zb1[e, fc*P+p]
    zb1_d = nc.dram_tensor("zb1r", [E * P, FC], F32, kind="ExternalInput")
    # zw28[(e*2+dh)*P+p, fcp, i, j] = 64*zw2[e, (2*fcp+i)*P+p, dh*512+j]
    zw28_d = nc.dram_tensor("zw28", [E * 2 * P, FC // 2, 2, 512], FP8,
                            kind="ExternalInput")
    zb2_d = nc.dram_tensor("zb2", [E, D], F32, kind="ExternalInput")
    out_d = nc.dram_tensor("out", [S, D], F32, kind="ExternalOutput")

    with tile.TileContext(nc) as tc:
        with (
            tc.tile_pool(name="const", bufs=1) as const,
            tc.tile_pool(name="xb", bufs=1) as xb,
            tc.tile_pool(name="wstream", bufs=1) as wstream,
            tc.tile_pool(name="ps", bufs=8, space="PSUM") as ps,
        ):
            # ---------- latency-critical loads, split across both rings ----------
            xT8all = xb.tile([P, DP, 2, S], FP8, name="xT8all")
            xt8v = xt8_d.rearrange("(c p) i s -> p c i s", p=P)
            nc.sync.dma_start(out=xT8all[:, 0:2], in_=xt8v[:, 0:2])
            nc.scalar.dma_start(out=xT8all[:, 2:4], in_=xt8v[:, 2:4])
            xT8 = [xT8all[:, dcp, :, :] for dcp in range(DP)]

            rw1_sb = const.tile([P, DC, H], BF16, name="rw1_sb")
            rw1v = rw1_d.rearrange("(c p) h -> p c h", p=P)
            nc.sync.dma_start(out=rw1_sb[:, 0:4, :], in_=rw1v[:, 0:4, :])
            nc.scalar.dma_start(out=rw1_sb[:, 4:8, :], in_=rw1v[:, 4:8, :])

            # router consts (scalar ring, small)
            rb1t_sb = const.tile([P, 2], F32, name="rb1t_sb")
            nc.scalar.dma_start(out=rb1t_sb, in_=rb1_d.rearrange("(i p) -> p i", p=P))
            rw2c_sb = const.tile([P, 2, E], F32, name="rw2c_sb")
            nc.scalar.dma_start(out=rw2c_sb, in_=rw2_d.rearrange("(i p) e -> p i e", p=P))
            rb2_sb = const.tile([1, E], F32, name="rb2_sb")
            nc.scalar.dma_start(out=rb2_sb, in_=rb2_d.rearrange("(o e) -> o e", o=1))
            zb2_sb = const.tile([1, E, D], F32, name="zb2_sb")
            nc.scalar.dma_start(out=zb2_sb, in_=zb2_d.rearrange("(o e) d -> o e d", o=1))

            # x chunks (residual): needed only for zacc init, after the consts
            x_all = xb.tile([P, TC, D], BF16, name="x_all")
            xv = x_d.rearrange("(t p) d -> p t d", p=P)
            nc.scalar.dma_start(out=x_all, in_=xv)

            onesb = const.tile([P, 1], FP8, name="onesb")
            nc.vector.memset(onesb, 1.0)

            # ---------- PE warm-up part 1: fill the pre-router idle ----------
            warm = ps.tile([1, 512], F32, name="warm", tag="warm", bufs=1)
            for i in range(16):
                nc.tensor.matmul(warm, onesb, xT8all[:, 0, 0, :],
                                 start=(i == 0), stop=(i == 15))

            # ---------- router: pooled reduce from fp8 xT8, h on the PE ----------
            # chunks 0..3 reduce on the DVE, 4..7 on the ACT engine (accum_out)
            pooled_f = const.tile([P, DC], F32, name="pooled_f")
            pooled_col = const.tile([P, DC], BF16, name="pooled_col")
            pscr = const.tile([P, S], BF16, name="pscr")
            for dc in range(4):
                nc.vector.tensor_reduce(out=pooled_f[:, dc:dc + 1],
                                        in_=xT8all[:, dc // 2, dc % 2, :],
                                        axis=mybir.AxisListType.X,
                                        op=mybir.AluOpType.add)
            for dc in range(4, DC):
                nc.scalar.activation(out=pscr, in_=xT8all[:, dc // 2, dc % 2, :],
                                     func=mybir.ActivationFunctionType.Copy,
                                     accum_out=pooled_f[:, dc:dc + 1])
            phT = [ps.tile([P, 1], F32, name=f"phT{i}", tag="phT", bufs=2)
                   for i in range(2)]
            for dc in range(DC):
                nc.vector.tensor_copy(out=pooled_col[:, dc:dc + 1],
                                      in_=pooled_f[:, dc:dc + 1])
                for i in range(2):
                    nc.tensor.matmul(phT[i], rw1_sb[:, dc, i * P:(i + 1) * P],
                                     pooled_col[:, dc:dc + 1],
                                     start=(dc == 0), stop=(dc == DC - 1))
            hpreT = const.tile([P, 2], F32, name="hpreT")
            for i in range(2):
                nc.vector.scalar_tensor_tensor(out=hpreT[:, i:i + 1], in0=phT[i],
                                               scalar=1.0 / S,
                                               in1=rb1t_sb[:, i:i + 1],
                                               op0=mybir.AluOpType.mult,
                                               op1=mybir.AluOpType.add)
            hT = const.tile([P, 2], F32, name="hT")
            nc.scalar.activation(out=hT, in_=hpreT,
                                 func=mybir.ActivationFunctionType.Tanh)
            lg = ps.tile([1, E], F32, name="lg", tag="warm", bufs=1)
            for i in range(2):
                nc.tensor.matmul(lg, hT[:, i:i + 1], rw2c_sb[:, i, :],
                                 start=(i == 0), stop=(i == 1))
            logits = const.tile([1, E], F32, name="logits")
            nc.vector.tensor_add(logits, lg, rb2_sb)

            # ---------- PE warm-up part 2: bridge the selection window ----------
            warm2 = ps.tile([1, 512], F32, name="warm2", tag="warm", bufs=1)
            for i in range(12):
                nc.tensor.matmul(warm2, onesb, xT8all[:, 0, 0, :],
                                 start=(i == 0), stop=(i == 11))

            # ---------- selection: dropped expert = argmin(logits) ----------
            lmin = const.tile([1, 1], F32, name="lmin")
            nc.vector.tensor_reduce(out=lmin, in_=logits, axis=mybir.AxisListType.X,
                                    op=mybir.AluOpType.min)
            iota4 = const.tile([1, E], F32, name="iota4")
            for e in range(E):
                nc.vector.memset(iota4[:, e:e + 1], float(e))
            lemask = const.tile([1, E], F32, name="lemask")
            nc.vector.tensor_scalar(out=lemask, in0=logits, scalar1=lmin, scalar2=None,
                                    op0=mybir.AluOpType.is_le)
            emul = const.tile([1, E], F32, name="emul")
            nc.vector.tensor_mul(emul, iota4, lemask)
            dminf = const.tile([1, 1], F32, name="dminf")
            nc.vector.tensor_reduce(out=dminf, in_=emul, axis=mybir.AxisListType.X,
                                    op=mybir.AluOpType.add)
            iota3 = const.tile([1, K], F32, name="iota3")
            iota3P = const.tile([1, K], F32, name="iota3P")
            iota3F2 = const.tile([1, K], F32, name="iota3F2")
            for k in range(K):
                nc.vector.memset(iota3[:, k:k + 1], float(k))
                nc.vector.memset(iota3P[:, k:k + 1], float(k * P))
                nc.vector.memset(iota3F2[:, k:k + 1], float(k * 2 * P))
            # ek[k] = k + (k >= dropped); ekP = 128*ek, ekF2 = 256*ek
            gemask = const.tile([1, K], F32, name="gemask")
            nc.vector.tensor_scalar(out=gemask, in0=iota3, scalar1=dminf, scalar2=None,
                                    op0=mybir.AluOpType.is_ge)
            ekP_f = const.tile([1, K], F32, name="ekP_f")
            nc.vector.scalar_tensor_tensor(out=ekP_f, in0=gemask, scalar=float(P),
                                           in1=iota3P, op0=mybir.AluOpType.mult,
                                           op1=mybir.AluOpType.add)
            ekP_i = const.tile([1, K], I32, name="ekP_i")
            nc.vector.tensor_copy(out=ekP_i, in_=ekP_f)
            ekF2_f = const.tile([1, K], F32, name="ekF2_f")
            nc.vector.scalar_tensor_tensor(out=ekF2_f, in0=gemask,
                                           scalar=float(2 * P), in1=iota3F2,
                                           op0=mybir.AluOpType.mult,
                                           op1=mybir.AluOpType.add)
            ekF2_i = const.tile([1, K], I32, name="ekF2_i")
            nc.vector.tensor_copy(out=ekF2_i, in_=ekF2_f)

            # sync ring: ekP (zw1 stream); scalar ring: ekP (zb1) + ekF2 (zw28)
            ekP_sy, ekP_sc, ekF2_sc = [], [], []
            for k in range(K):
                rP = nc.sync.alloc_register(f"rP_sy{k}")
                nc.reg_load(rP, ekP_i[:, k:k + 1])
                ekP_sy.append(nc.sync.snap(rP))
                rPs = nc.scalar.alloc_register(f"rP_sc{k}")
                nc.reg_load(rPs, ekP_i[:, k:k + 1])
                ekP_sc.append(nc.scalar.snap(rPs))
                rFs = nc.scalar.alloc_register(f"rF_sc{k}")
                nc.reg_load(rFs, ekF2_i[:, k:k + 1])
                ekF2_sc.append(nc.scalar.snap(rFs))



            # ---------- expert pipeline state ----------
            wbc3 = const.tile([P, 1, K], F32, name="wbc3")
            wbc = wbc3[:, 0, :]
            zacc = [xb.tile([P, D], F32, name=f"zacc{t}") for t in range(TC)]
            hid8 = [xb.tile([P, FC, S], FP8, name=f"hid8{i}") for i in range(2)]

            zw1q = {}

            def load_zw1(k, fcp, split=False):
                # one tile covers fc chunks 2*fcp, 2*fcp+1
                t = wstream.tile([P, 2, DP, 2, P], FP8, name=f"zw1q{k}_{fcp}",
                                 tag="zw1q", bufs=PRE1 + 2)
                if split:
                    nc.sync.dma_start(out=t[:, 0],
                                      in_=zw1_d[bass.ds(ekP_sy[k], P), fcp, 0])
                    nc.scalar.dma_start(out=t[:, 1],
                                        in_=zw1_d[bass.ds(ekP_sc[k], P), fcp, 1])
                else:
                    nc.sync.dma_start(out=t, in_=zw1_d[bass.ds(ekP_sy[k], P), fcp])
                zw1q[(k, fcp)] = t

            zw28q = {}

            def load_zw28(k, dh, fch, half):
                # one quarter (4 fc pairs = one drain-quantum half): 512KB,
                # bounds the DMA queue-FIFO backlog seen by the zw1 stream
                lo = fch * 8 + half * 4
                t = wstream.tile([P, 4, 2, 512], FP8,
                                 name=f"zw28q{k}_{dh}_{fch}_{half}",
                                 tag="zw28q", bufs=NB28)
                nc.scalar.dma_start(
                    out=t, in_=zw28_d[bass.ds(ekF2_sc[k] + dh * P, P),
                                      lo:lo + 4])
                zw28q[(k, dh, fch, half)] = t

            g2_state = {}

            def emit_g2_quantum(k, dh, fch, t, half):
                key = (k, dh, fch, t)
                if key not in g2_state:
                    g2_state[key] = ps.tile([P, 512], F32,
                                            name=f"p2_{k}_{dh}_{fch}_{t}",
                                            tag="p2", bufs=2)
                p2 = g2_state[key]
                h8 = hid8[k % 2]
                w8 = zw28q[(k, dh, fch, half)]
                for q in range(4):
                    fcp = fch * 8 + half * 4 + q
                    nc.tensor.matmul(p2, h8[:, 2 * fcp:2 * fcp + 2,
                                           t * P:(t + 1) * P],
                                     w8[:, q, :, :],
                                     start=(half == 0 and q == 0),
                                     stop=(half == 1 and q == 3),
                                     perf_mode=DR)
                if half == 1:
                    # evict this fch's partial accumulation into zacc (additive);
                    # wbc carries the 1/W2SCALE compensation
                    sl = slice(dh * 512, (dh + 1) * 512)
                    nc.vector.scalar_tensor_tensor(
                        out=zacc[t][:, sl], in0=p2, scalar=wbc[:, k:k + 1],
                        in1=zacc[t][:, sl], op0=mybir.AluOpType.mult,
                        op1=mybir.AluOpType.add)
                    if k == K - 1 and fch == 1 and dh == 1:
                        # zacc[t] is now final across both d-halves; both on the
                        # sync ring (idle here) to keep the GELU stream clean
                        nc.sync.dma_start(out=out_d[t * P:(t + 1) * P, 0:512],
                                          in_=zacc[t][:, 0:512])
                        nc.sync.dma_start(out=out_d[t * P:(t + 1) * P, 512:1024],
                                          in_=zacc[t][:, 512:1024])

            def g2_quanta():
                for k in range(K):
                    for fch in range(2):
                        if k == K - 1 and fch == 1:
                            # t-major so each zacc[t] finalizes (and DMAs out)
                            # as early as possible through the tail
                            for t in range(TC):
                                for dh in range(2):
                                    for half in range(2):
                                        yield (k, dh, fch, t, half)
                        else:
                            for dh in range(2):
                                for t in range(TC):
                                    for half in range(2):
                                        yield (k, dh, fch, t, half)

            g2_iter = iter(g2_quanta())

            def drain_g2(n=1):
                for _ in range(n):
                    q = next(g2_iter, None)
                    if q is not None:
                        emit_g2_quantum(*q)

            # prologue prefetches: first zw1 tiles lead both rings so GEMM1
            # can start the moment the register snaps resolve
            zb1_sb = [wstream.tile([P, FC], F32, name=f"zb1_sb{k}",
                                   tag="zb1", bufs=K) for k in range(K)]
            nc.scalar.dma_start(out=zb1_sb[0], in_=zb1_d[bass.ds(ekP_sc[0], P), :])
            for fcp in range(4):
                load_zw1(0, fcp, split=True)
            for k in range(1, K):
                nc.scalar.dma_start(out=zb1_sb[k],
                                    in_=zb1_d[bass.ds(ekP_sc[k], P), :])
            for fcp in range(4, PRE1):
                load_zw1(0, fcp)

            # zw28 quarter prefetches: 8 per expert in first-use order
            # (dh0f0h0, dh0f0h1, dh1f0h0, dh1f0h1, dh0f1h0, ...), issued at
            # these fc points of each expert's GEMM1 loop (~14 steps of lead)
            q_order = [(0, 0, 0), (0, 0, 1), (1, 0, 0), (1, 0, 1),
                       (0, 1, 0), (0, 1, 1), (1, 1, 0), (1, 1, 1)]
            q_points = [2, 4, 8, 10, 16, 18, 24, 26]
            zw28_sched = {}
            for k in range(K):
                for (dh_, fch_, h_), fc_ in zip(q_order, q_points):
                    zw28_sched[(k, fc_)] = (k, dh_, fch_, h_)

            # ---------- combine weights (off critical path) ----------
            mx = const.tile([1, 1], F32, name="mx")
            nc.vector.tensor_reduce(out=mx, in_=logits, axis=mybir.AxisListType.X,
                                    op=mybir.AluOpType.max)
            sh = const.tile([1, E], F32, name="sh")
            nc.vector.tensor_scalar(out=sh, in0=logits, scalar1=mx,
                                    scalar2=None, op0=mybir.AluOpType.subtract)
            ex = const.tile([1, E], F32, name="ex")
            nc.scalar.activation(out=ex, in_=sh,
                                 func=mybir.ActivationFunctionType.Exp)
            sm = const.tile([1, 1], F32, name="sm")
            nc.vector.tensor_reduce(out=sm, in_=ex, axis=mybir.AxisListType.X,
                                    op=mybir.AluOpType.add)
            rs = const.tile([1, 1], F32, name="rs")
            nc.vector.reciprocal(out=rs, in_=sm)
            probs = const.tile([1, E], F32, name="probs")
            nc.vector.tensor_scalar(out=probs, in0=ex, scalar1=rs, scalar2=None,
                                    op0=mybir.AluOpType.mult)
            pmin = const.tile([1, 1], F32, name="pmin")
            nc.vector.tensor_reduce(out=pmin, in_=probs, axis=mybir.AxisListType.X,
                                    op=mybir.AluOpType.min)
            onec = const.tile([1, 1], F32, name="onec")
            nc.vector.memset(onec, 1.0)
            den = const.tile([1, 1], F32, name="den")
            nc.vector.tensor_sub(den, onec, pmin)
            rden = const.tile([1, 1], F32, name="rden")
            nc.vector.reciprocal(out=rden, in_=den)
            gtmask = const.tile([1, E], F32, name="gtmask")
            nc.vector.tensor_scalar(out=gtmask, in0=probs, scalar1=pmin,
                                    scalar2=None, op0=mybir.AluOpType.is_gt)
            wall = const.tile([1, E], F32, name="wall")
            nc.vector.tensor_mul(wall, probs, gtmask)
            w_sb = const.tile([1, E], F32, name="w_sb")
            nc.vector.tensor_scalar(out=w_sb, in0=wall, scalar1=rden,
                                    scalar2=None, op0=mybir.AluOpType.mult)
            wdiff = const.tile([1, K], F32, name="wdiff")
            nc.vector.tensor_sub(wdiff, w_sb[:, 1:E], w_sb[:, 0:K])
            wstep = const.tile([1, K], F32, name="wstep")
            nc.vector.tensor_mul(wstep, wdiff, gemask)
            wc = const.tile([1, K], F32, name="wc")
            nc.vector.tensor_add(wc, w_sb[:, 0:K], wstep)
            wc64 = const.tile([1, K], F32, name="wc64")
            nc.vector.tensor_scalar(out=wc64, in0=wc, scalar1=1.0 / W2SCALE,
                                    scalar2=None, op0=mybir.AluOpType.mult)
            nc.gpsimd.partition_broadcast(wbc3[:, 0, :], wc64, channels=P)
            zb2sum = const.tile([1, D], F32, name="zb2sum")
            nc.vector.tensor_scalar(out=zb2sum, in0=zb2_sb[:, 0, :],
                                    scalar1=w_sb[:, 0:1], scalar2=None,
                                    op0=mybir.AluOpType.mult)
            for e in range(1, E):
                nc.vector.scalar_tensor_tensor(out=zb2sum, in0=zb2_sb[:, e, :],
                                               scalar=w_sb[:, e:e + 1], in1=zb2sum,
                                               op0=mybir.AluOpType.mult,
                                               op1=mybir.AluOpType.add)
            zb2b3 = const.tile([P, 1, D], F32, name="zb2b3")
            nc.gpsimd.partition_broadcast(zb2b3[:, 0, :], zb2sum, channels=P)
            for t in range(TC):
                nc.vector.scalar_tensor_tensor(out=zacc[t], in0=x_all[:, t, :],
                                               scalar=1.0, in1=zb2b3[:, 0, :],
                                               op0=mybir.AluOpType.mult,
                                               op1=mybir.AluOpType.add)

            # preload the GELU activation table after the softmax Exp (the
            # w_sb bias forces exp -> gelu order on the scalar stream)
            gelu_warm = const.tile([1, DC], F32, name="gelu_warm")
            nc.scalar.activation(out=gelu_warm, in_=pooled_f[0:1, :], func=GELU,
                                 bias=w_sb[:, 0:1])

            # ---------- main loop ----------
            NPAIR = FC // 2
            for k in range(K):
                for fc in range(FC):
                    if fc % 2 == 0:
                        nfcp = fc // 2 + PRE1
                        if nfcp < NPAIR:
                            load_zw1(k, nfcp)
                        elif k + 1 < K:
                            load_zw1(k + 1, nfcp - NPAIR)
                    nxt28 = zw28_sched.get((k, fc))
                    if nxt28 is not None:
                        load_zw28(*nxt28)
                    p1 = ps.tile([P, S], F32, name=f"p1_{k}_{fc}", tag="p1", bufs=3)
                    w1t = zw1q[(k, fc // 2)]
                    for dcp in range(DP):
                        nc.tensor.matmul(p1, w1t[:, fc % 2, dcp, :, :], xT8[dcp],
                                         start=(dcp == 0), stop=(dcp == DP - 1),
                                         perf_mode=DR)
                    nc.scalar.activation(out=hid8[k % 2][:, fc, :], in_=p1,
                                         func=GELU, bias=zb1_sb[k][:, fc:fc + 1],
                                         scale=1.0 / W1SCALE)
                    if k > 0 or fc >= 16:
                        drain_g2(1)
            # tail: drain remaining G2 work (expert 2 fch=1 groups)
            drain_g2(16)

    nc.finalize()
    return nc


_NC_CACHE = None


def _get_nc():
    global _NC_CACHE
    if _NC_CACHE is None:
        _NC_CACHE = build_nc()
    return _NC_CACHE


def kernel(x, rw1, rb1, rw2, rb2, zw1, zb1, zw2, zb2, **run_kwargs):
    x = np.asarray(x, dtype=np.float32)
    zw1 = np.asarray(zw1, np.float32)
    zw2 = np.asarray(zw2, np.float32)
    zb1 = np.asarray(zb1, np.float32)
    # zw1r8[e*P+p, fcp, j, dcp, i, fw] = 32*zw1[e, (2*dcp+i)*P+p, (2*fcp+j)*P+fw]
    zw1r = np.ascontiguousarray(
        (zw1 * W1SCALE).reshape(E, DP, 2, P, FC // 2, 2, P)
        .transpose(0, 3, 4, 5, 1, 2, 6)
        .reshape(E * P, FC // 2, 2, DP, 2, P)).astype(ml_dtypes.float8_e4m3)
    zb1r = np.ascontiguousarray(
        zb1.reshape(E, FC, P).transpose(0, 2, 1).reshape(E * P, FC))
    # zw28[(e*2+dh)*P+p, fcp, i, j] = 64*zw2[e, (2*fcp+i)*P+p, dh*512+j]
    zw28 = np.ascontiguousarray(
        (zw2 * W2SCALE).reshape(E, FC // 2, 2, P, 2, 512)
        .transpose(0, 4, 3, 1, 2, 5)
        .reshape(E * 2 * P, FC // 2, 2, 512)).astype(ml_dtypes.float8_e4m3)
    shared = {
        "rw1b": np.asarray(rw1, np.float32).astype(ml_dtypes.bfloat16),
        "rb1": np.ascontiguousarray(np.asarray(rb1, np.float32)),
        "rw2": np.ascontiguousarray(np.asarray(rw2, np.float32)),
        "rb2": np.ascontiguousarray(np.asarray(rb2, np.float32)),
        "zw1r": zw1r,
        "zb1r": zb1r,
        "zw28": zw28,
        "zb2": np.ascontiguousarray(np.asarray(zb2, np.float32)),
    }
    B = x.shape[0]
    nc = _get_nc()
    in_maps = []
    for b in range(B):
        xb_ = x[b]
        xt = np.ascontiguousarray(xb_.T)            # [D, S] fp32
        m = dict(shared, x=xb_.astype(ml_dtypes.bfloat16),
                 xt8=np.ascontiguousarray(
                     xt.reshape(DP, 2, P, S).transpose(0, 2, 1, 3)
                     .reshape(DP * P, 2, S)).astype(ml_dtypes.float8_e4m3))
        in_maps.append(m)
    res = run_bass_kernel_spmd(nc, in_maps, core_ids=list(range(B)), **run_kwargs)
    out = np.stack([res.results[b]["out"] for b in range(B)], axis=0)
    if run_kwargs:
        kernel.last_results = res
    return out


if __name__ == "__main__":
    rng = np.random.default_rng(0)
    inputs = {
        "x": rng.standard_normal((8, S, D)).astype(np.float32),
        "rw1": (rng.standard_normal((D, H)) / np.sqrt(D)).astype(np.float32),
        "rb1": np.zeros(H, np.float32),
        "rw2": (rng.standard_normal((H, E)) / np.sqrt(H)).astype(np.float32),
        "rb2": np.zeros(E, np.float32),
        "zw1": (rng.standard_normal((E, D, F)) / np.sqrt(D)).astype(np.float32),
        "zb1": np.zeros((E, F), np.float32),
        "zw2": (rng.standard_normal((E, F, D)) / np.sqrt(F)).astype(np.float32),
        "zb2": np.zeros((E, D), np.float32),
    }
    out = kernel(**inputs)
    print("out", out.shape, out.dtype, np.abs(out).max())


# revision 23
# speedup vs baseline: 1.0011x; 1.0011x over previous
"""MoE routing kernel for Trainium2 (8 NeuronCores, batch-parallel), v10.

Per batch element b (one NeuronCore each):
    pooled = mean_s x[b]; h = tanh(pooled @ rw1 + rb1)
    logits = h @ rw2 + rb2; probs = softmax(logits)
    top-3 of 4 experts, renormalized; out[b] = x[b] + sum_e w[e] * z_e

Design (206.5us worst-core, rel err 1.948e-2):
  - Both GEMMs entirely fp8e4m3 DoubleRow (157 TF/s). Afforded by scaling
    zw1 x32 and zw2 x64 host-side before fp8 cast (pulls sigma~1/32,
    1/64 weights out of the e4m3 subnormal range); compensated exactly
    by GELU scale=1/32 and combine-weights/64. No bf16 zw2 stream.
  - Router pools from the fp8 xT8 tiles (selection margin ~20x the fp8
    logit perturbation); no separate bf16 transposed x copy. Pooled
    reduce split DVE (chunks 0-3) / ACT accum_out (4-7).
  - Startup: xt8 + rw1 split across both rings and issued first; x
    (residual) deferred behind the router consts; first 4 zw1 tiles
    split across rings; scalar ring has its own register snaps and
    carries zb1 + the zw28 stream. Warm MMs bracket the router PE work.
  - zw28 streamed as 512KB quarter-tiles (one per drain-quantum half) on
    a spread schedule so the zw1 tile stream never sits behind >1.4us of
    bulk in the DMA queue FIFOs; p1 PSUM bufs=3 absorbs GELU jitter.
  - gelu_warm takes a softmax-dependent bias so the scalar stream is
    tanh -> exp -> gelu (no GELU table reload mid-stream).
  - GEMM2 drain lags GEMM1 by 16 fc steps; last expert's fch=1 block is
    t-major so each zacc[t] finalizes early and flows out as two 256KB
    DMAs on the (idle) sync ring through the tail.
"""
import sys

sys.path.insert(0, "/opt/trn_rl_repo")

import numpy as np
import ml_dtypes

import concourse.bacc as bacc
import concourse.bass as bass
import concourse.mybir as mybir
import concourse.tile as tile
from concourse.bass_utils import run_bass_kernel_spmd

S, D, F, E, H = 512, 1024, 4096, 4, 256
K = 3            # active experts (top-3 of 4)
P = 128
TC = S // P      # 4 token chunks
DC = D // P      # 8 d chunks
DP = DC // 2     # 4 d chunk-pairs (DoubleRow K=256)
FC = F // P      # 32 ff chunks
F32 = mybir.dt.float32
BF16 = mybir.dt.bfloat16
FP8 = mybir.dt.float8e4
I32 = mybir.dt.int32
GELU = mybir.ActivationFunctionType.Gelu_apprx_tanh
DR = mybir.MatmulPerfMode.DoubleRow
W1SCALE = 32.0   # zw1 pre-scale (host) / GELU scale compensation
W2SCALE = 64.0   # zw2 pre-scale (host) / combine-weight compensation
PRE1 = 11        # zw1 pair-tile prefetch depth (covers 2*PRE1 fc chunks)
NB28 = 4         # zw28 half-tile pool size (8KB/partition each)


def build_nc():
    nc = bacc.Bacc("TRN2", target_bir_lowering=False, debug=False)

    x_d = nc.dram_tensor("x", [S, D], BF16, kind="ExternalInput")
    xt8_d = nc.dram_tensor("xt8", [DP * P, 2, S], FP8, kind="ExternalInput")
    rw1_d = nc.dram_tensor("rw1b", [D, H], BF16, kind="ExternalInput")
    rb1_d = nc.dram_tensor("rb1", [H], F32, kind="ExternalInput")
    rw2_d = nc.dram_tensor("rw2", [H, E], F32, kind="ExternalInput")
    rb2_d = nc.dram_tensor("rb2", [E], F32, kind="ExternalInput")
    # zw1r8[e*P+p, fcp, j, dcp, i, fw] = 32*zw1[e, (2*dcp+i)*P+p, (2*fcp+j)*P+fw]
    zw1_d = nc.dram_tensor("zw1r", [E * P, FC // 2, 2, DP, 2, P], FP8,
                           kind="ExternalInput")
    # zb1r[e*P+p, fc] = zb1[e, fc*P+p]
    zb1_d = nc.dram_tensor("zb1r", [E * P, FC], F32, kind="ExternalInput")
    # zw28[(e*2+dh)*P+p, fcp, i, j] = 64*zw2[e, (2*fcp+i)*P+p, dh*512+j]
    zw28_d = nc.dram_tensor("zw28", [E * 2 * P, FC // 2, 2, 512], FP8,
                            kind="ExternalInput")
    zb2_d = nc.dram_tensor("zb2", [E, D], F32, kind="ExternalInput")
    out_d = nc.dram_tensor("out", [S, D], F32, kind="ExternalOutput")

    with tile.TileContext(nc) as tc:
        with (
            tc.tile_pool(name="const", bufs=1) as const,
            tc.tile_pool(name="xb", bufs=1) as xb,
            tc.tile_pool(name="wstream", bufs=1) as wstream,
            tc.tile_pool(name="ps", bufs=8, space="PSUM") as ps,
        ):
            # ---------- latency-critical loads, split across both rings ----------
            xT8all = xb.tile([P, DP, 2, S], FP8, name="xT8all")
            xt8v = xt8_d.rearrange("(c p) i s -> p c i s", p=P)
            nc.sync.dma_start(out=xT8all[:, 0:2], in_=xt8v[:, 0:2])
            nc.scalar.dma_start(out=xT8all[:, 2:4], in_=xt8v[:, 2:4])
            xT8 = [xT8all[:, dcp, :, :] for dcp in range(DP)]

            rw1_sb = const.tile([P, DC, H], BF16, name="rw1_sb")
            rw1v = rw1_d.rearrange("(c p) h -> p c h", p=P)
            nc.sync.dma_start(out=rw1_sb[:, 0:4, :], in_=rw1v[:, 0:4, :])
            nc.scalar.dma_start(out=rw1_sb[:, 4:8, :], in_=rw1v[:, 4:8, :])

            # router consts (scalar ring, small)
            rb1t_sb = const.tile([P, 2], F32, name="rb1t_sb")
            nc.scalar.dma_start(out=rb1t_sb, in_=rb1_d.rearrange("(i p) -> p i", p=P))
            rw2c_sb = const.tile([P, 2, E], F32, name="rw2c_sb")
            nc.scalar.dma_start(out=rw2c_sb, in_=rw2_d.rearrange("(i p) e -> p i e", p=P))
            rb2_sb = const.tile([1, E], F32, name="rb2_sb")
            nc.scalar.dma_start(out=rb2_sb, in_=rb2_d.rearrange("(o e) -> o e", o=1))
            zb2_sb = const.tile([1, E, D], F32, name="zb2_sb")
            nc.scalar.dma_start(out=zb2_sb, in_=zb2_d.rearrange("(o e) d -> o e d", o=1))

            # x chunks (residual): needed only for zacc init, after the consts
            x_all = xb.tile([P, TC, D], BF16, name="x_all")
            xv = x_d.rearrange("(t p) d -> p t d", p=P)
            nc.scalar.dma_start(out=x_all, in_=xv)

            onesb = const.tile([P, 1], FP8, name="onesb")
            nc.vector.memset(onesb, 1.0)

            # ---------- PE warm-up part 1: fill the pre-router idle ----------
            warm = ps.tile([1, 512], F32, name="warm", tag="warm", bufs=1)
            for i in range(16):
                nc.tensor.matmul(warm, onesb, xT8all[:, 0, 0, :],
                                 start=(i == 0), stop=(i == 15))

            # ---------- router: pooled reduce from fp8 xT8, h on the PE ----------
            # chunks 0..3 reduce on the DVE, 4..7 on the ACT engine (accum_out)
            pooled_f = const.tile([P, DC], F32, name="pooled_f")
            pooled_col = const.tile([P, DC], BF16, name="pooled_col")
            pscr = const.tile([P, S], BF16, name="pscr")
            for dc in range(4):
                nc.vector.tensor_reduce(out=pooled_f[:, dc:dc + 1],
                                        in_=xT8all[:, dc // 2, dc % 2, :],
                                        axis=mybir.AxisListType.X,
                                        op=mybir.AluOpType.add)
            for dc in range(4, DC):
                nc.scalar.activation(out=pscr, in_=xT8all[:, dc // 2, dc % 2, :],
                                     func=mybir.ActivationFunctionType.Copy,
                                     accum_out=pooled_f[:, dc:dc + 1])
            phT = [ps.tile([P, 1], F32, name=f"phT{i}", tag="phT", bufs=2)
                   for i in range(2)]
            for dc in range(DC):
                nc.vector.tensor_copy(out=pooled_col[:, dc:dc + 1],
                                      in_=pooled_f[:, dc:dc + 1])
                for i in range(2):
                    nc.tensor.matmul(phT[i], rw1_sb[:, dc, i * P:(i + 1) * P],
                                     pooled_col[:, dc:dc + 1],
                                     start=(dc == 0), stop=(dc == DC - 1))
            hpreT = const.tile([P, 2], F32, name="hpreT")
            for i in range(2):
                nc.vector.scalar_tensor_tensor(out=hpreT[:, i:i + 1], in0=phT[i],
                                               scalar=1.0 / S,
                                               in1=rb1t_sb[:, i:i + 1],
                                               op0=mybir.AluOpType.mult,
                                               op1=mybir.AluOpType.add)
            hT = const.tile([P, 2], F32, name="hT")
            nc.scalar.activation(out=hT, in_=hpreT,
                                 func=mybir.ActivationFunctionType.Tanh)
            lg = ps.tile([1, E], F32, name="lg", tag="warm", bufs=1)
            for i in range(2):
                nc.tensor.matmul(lg, hT[:, i:i + 1], rw2c_sb[:, i, :],
                                 start=(i == 0), stop=(i == 1))
            logits = const.tile([1, E], F32, name="logits")
            nc.vector.tensor_add(logits, lg, rb2_sb)

            # ---------- PE warm-up part 2: bridge the selection window ----------
            warm2 = ps.tile([1, 512], F32, name="warm2", tag="warm", bufs=1)
            for i in range(12):
                nc.tensor.matmul(warm2, onesb, xT8all[:, 0, 0, :],
                                 start=(i == 0), stop=(i == 11))

            # ---------- selection: dropped expert = argmin(logits) ----------
            lmin = const.tile([1, 1], F32, name="lmin")
            nc.vector.tensor_reduce(out=lmin, in_=logits, axis=mybir.AxisListType.X,
                                    op=mybir.AluOpType.min)
            iota4 = const.tile([1, E], F32, name="iota4")
            for e in range(E):
                nc.vector.memset(iota4[:, e:e + 1], float(e))
            lemask = const.tile([1, E], F32, name="lemask")
            nc.vector.tensor_scalar(out=lemask, in0=logits, scalar1=lmin, scalar2=None,
                                    op0=mybir.AluOpType.is_le)
            emul = const.tile([1, E], F32, name="emul")
            nc.vector.tensor_mul(emul, iota4, lemask)
            dminf = const.tile([1, 1], F32, name="dminf")
            nc.vector.tensor_reduce(out=dminf, in_=emul, axis=mybir.AxisListType.X,
                                    op=mybir.AluOpType.add)
            iota3 = const.tile([1, K], F32, name="iota3")
            iota3P = const.tile([1, K], F32, name="iota3P")
            iota3F2 = const.tile([1, K], F32, name="iota3F2")
            for k in range(K):
                nc.vector.memset(iota3[:, k:k + 1], float(k))
                nc.vector.memset(iota3P[:, k:k + 1], float(k * P))
                nc.vector.memset(iota3F2[:, k:k + 1], float(k * 2 * P))
            # ek[k] = k + (k >= dropped); ekP = 128*ek, ekF2 = 256*ek
            gemask = const.tile([1, K], F32, name="gemask")
            nc.vector.tensor_scalar(out=gemask, in0=iota3, scalar1=dminf, scalar2=None,
                                    op0=mybir.AluOpType.is_ge)
            ekP_f = const.tile([1, K], F32, name="ekP_f")
            nc.vector.scalar_tensor_tensor(out=ekP_f, in0=gemask, scalar=float(P),
                                           in1=iota3P, op0=mybir.AluOpType.mult,
                                           op1=mybir.AluOpType.add)
            ekP_i = const.tile([1, K], I32, name="ekP_i")
            nc.vector.tensor_copy(out=ekP_i, in_=ekP_f)
            ekF2_f = const.tile([1, K], F32, name="ekF2_f")
            nc.vector.scalar_tensor_tensor(out=ekF2_f, in0=gemask,
                                           scalar=float(2 * P), in1=iota3F2,
                                           op0=mybir.AluOpType.mult,
                                           op1=mybir.AluOpType.add)
            ekF2_i = const.tile([1, K], I32, name="ekF2_i")
            nc.vector.tensor_copy(out=ekF2_i, in_=ekF2_f)

            # sync ring: ekP (zw1 stream); scalar ring: ekP (zb1) + ekF2 (zw28)
            ekP_sy, ekP_sc, ekF2_sc = [], [], []
            for k in range(K):
                rP = nc.sync.alloc_register(f"rP_sy{k}")
                nc.reg_load(rP, ekP_i[:, k:k + 1])
                ekP_sy.append(nc.sync.snap(rP))
                rPs = nc.scalar.alloc_register(f"rP_sc{k}")
                nc.reg_load(rPs, ekP_i[:, k:k + 1])
                ekP_sc.append(nc.scalar.snap(rPs))
                rFs = nc.scalar.alloc_register(f"rF_sc{k}")
                nc.reg_load(rFs, ekF2_i[:, k:k + 1])
                ekF2_sc.append(nc.scalar.snap(rFs))



            # ---------- expert pipeline state ----------
            wbc3 = const.tile([P, 1, K], F32, name="wbc3")
            wbc = wbc3[:, 0, :]
            zacc = [xb.tile([P, D], F32, name=f"zacc{t}") for t in range(TC)]
            hid8 = [xb.tile([P, FC, S], FP8, name=f"hid8{i}") for i in range(2)]

            zw1q = {}

            def load_zw1(k, fcp, split=False):
                # one tile covers fc chunks 2*fcp, 2*fcp+1
                t = wstream.tile([P, 2, DP, 2, P], FP8, name=f"zw1q{k}_{fcp}",
                                 tag="zw1q", bufs=PRE1 + 2)
                if split:
                    nc.sync.dma_start(out=t[:, 0],
                                      in_=zw1_d[bass.ds(ekP_sy[k], P), fcp, 0])
                    nc.scalar.dma_start(out=t[:, 1],
                                        in_=zw1_d[bass.ds(ekP_sc[k], P), fcp, 1])
                else:
                    nc.sync.dma_start(out=t, in_=zw1_d[bass.ds(ekP_sy[k], P), fcp])
                zw1q[(k, fcp)] = t

            zw28q = {}

            def load_zw28(k, dh, fch, half):
                # one quarter (4 fc pairs = one drain-quantum half): 512KB,
                # bounds the DMA queue-FIFO backlog seen by the zw1 stream
                lo = fch * 8 + half * 4
                t = wstream.tile([P, 4, 2, 512], FP8,
                                 name=f"zw28q{k}_{dh}_{fch}_{half}",
                                 tag="zw28q", bufs=NB28)
                nc.scalar.dma_start(
                    out=t, in_=zw28_d[bass.ds(ekF2_sc[k] + dh * P, P),
                                      lo:lo + 4])
                zw28q[(k, dh, fch, half)] = t

            g2_state = {}

            def emit_g2_quantum(k, dh, fch, t, half):
                key = (k, dh, fch, t)
                if key not in g2_state:
                    g2_state[key] = ps.tile([P, 512], F32,
                                            name=f"p2_{k}_{dh}_{fch}_{t}",
                                            tag="p2", bufs=2)
                p2 = g2_state[key]
                h8 = hid8[k % 2]
                w8 = zw28q[(k, dh, fch, half)]
                for q in range(4):
                    fcp = fch * 8 + half * 4 + q
                    nc.tensor.matmul(p2, h8[:, 2 * fcp:2 * fcp + 2,
                                           t * P:(t + 1) * P],
                                     w8[:, q, :, :],
                                     start=(half == 0 and q == 0),
                                     stop=(half == 1 and q == 3),
                                     perf_mode=DR)
                if half == 1:
                    # evict this fch's partial accumulation into zacc (additive);
                    # wbc carries the 1/W2SCALE compensation
                    sl = slice(dh * 512, (dh + 1) * 512)
                    nc.vector.scalar_tensor_tensor(
                        out=zacc[t][:, sl], in0=p2, scalar=wbc[:, k:k + 1],
                        in1=zacc[t][:, sl], op0=mybir.AluOpType.mult,
                        op1=mybir.AluOpType.add)
                    if k == K - 1 and fch == 1 and dh == 1:
                        # zacc[t] is now final across both d-halves; both on the
                        # sync ring (idle here) to keep the GELU stream clean
                        nc.sync.dma_start(out=out_d[t * P:(t + 1) * P, 0:512],
                                          in_=zacc[t][:, 0:512])
                        nc.sync.dma_start(out=out_d[t * P:(t + 1) * P, 512:1024],
                                          in_=zacc[t][:, 512:1024])

            def g2_quanta():
                for k in range(K):
                    for fch in range(2):
                        if k == K - 1 and fch == 1:
                            # t-major so each zacc[t] finalizes (and DMAs out)
                            # as early as possible through the tail
                            for t in range(TC):
                                for dh in range(2):
                                    for half in range(2):
                                        yield (k, dh, fch, t, half)
                        else:
                            for dh in range(2):
                                for t in range(TC):
                                    for half in range(2):
                                        yield (k, dh, fch, t, half)

            g2_iter = iter(g2_quanta())

            def drain_g2(n=1):
                for _ in range(n):
                    q = next(g2_iter, None)
                    if q is not None:
                        emit_g2_quantum(*q)

            # prologue prefetches: first zw1 tiles lead both rings so GEMM1
            # can start the moment the register snaps resolve
            zb1_sb = [wstream.tile([P, FC], F32, name=f"zb1_sb{k}",
                                   tag="zb1", bufs=K) for k in range(K)]
            nc.scalar.dma_start(out=zb1_sb[0], in_=zb1_d[bass.ds(ekP_sc[0], P), :])
            for fcp in range(4):
                load_zw1(0, fcp, split=True)
            for k in range(1, K):
                nc.scalar.dma_start(out=zb1_sb[k],
                                    in_=zb1_d[bass.ds(ekP_sc[k], P), :])
            for fcp in range(4, PRE1):
                load_zw1(0, fcp)

            # zw28 quarter prefetches: 8 per expert in first-use order
            # (dh0f0h0, dh0f0h1, dh1f0h0, dh1f0h1, dh0f1h0, ...), issued at
            # these fc points of each expert's GEMM1 loop (~14 steps of lead)
            q_order = [(0, 0, 0), (0, 0, 1), (1, 0, 0), (1, 0, 1),
                       (0, 1, 0), (0, 1, 1), (1, 1, 0), (1, 1, 1)]
            q_points = [2, 4, 8, 10, 16, 18, 24, 26]
            zw28_sched = {}
            for k in range(K):
                for (dh_, fch_, h_), fc_ in zip(q_order, q_points):
                    zw28_sched[(k, fc_)] = (k, dh_, fch_, h_)

            # ---------- combine weights (off critical path) ----------
            mx = const.tile([1, 1], F32, name="mx")
            nc.vector.tensor_reduce(out=mx, in_=logits, axis=mybir.AxisListType.X,
                                    op=mybir.AluOpType.max)
            sh = const.tile([1, E], F32, name="sh")
            nc.vector.tensor_scalar(out=sh, in0=logits, scalar1=mx,
                                    scalar2=None, op0=mybir.AluOpType.subtract)
            ex = const.tile([1, E], F32, name="ex")
            nc.scalar.activation(out=ex, in_=sh,
                                 func=mybir.ActivationFunctionType.Exp)
            sm = const.tile([1, 1], F32, name="sm")
            nc.vector.tensor_reduce(out=sm, in_=ex, axis=mybir.AxisListType.X,
                                    op=mybir.AluOpType.add)
            rs = const.tile([1, 1], F32, name="rs")
            nc.vector.reciprocal(out=rs, in_=sm)
            probs = const.tile([1, E], F32, name="probs")
            nc.vector.tensor_scalar(out=probs, in0=ex, scalar1=rs, scalar2=None,
                                    op0=mybir.AluOpType.mult)
            pmin = const.tile([1, 1], F32, name="pmin")
            nc.vector.tensor_reduce(out=pmin, in_=probs, axis=mybir.AxisListType.X,
                                    op=mybir.AluOpType.min)
            onec = const.tile([1, 1], F32, name="onec")
            nc.vector.memset(onec, 1.0)
            den = const.tile([1, 1], F32, name="den")
            nc.vector.tensor_sub(den, onec, pmin)
            rden = const.tile([1, 1], F32, name="rden")
            nc.vector.reciprocal(out=rden, in_=den)
            gtmask = const.tile([1, E], F32, name="gtmask")
            nc.vector.tensor_scalar(out=gtmask, in0=probs, scalar1=pmin,
                                    scalar2=None, op0=mybir.AluOpType.is_gt)
            wall = const.tile([1, E], F32, name="wall")
            nc.vector.tensor_mul(wall, probs, gtmask)
            w_sb = const.tile([1, E], F32, name="w_sb")
            nc.vector.tensor_scalar(out=w_sb, in0=wall, scalar1=rden,
                                    scalar2=None, op0=mybir.AluOpType.mult)
            wdiff = const.tile([1, K], F32, name="wdiff")
            nc.vector.tensor_sub(wdiff, w_sb[:, 1:E], w_sb[:, 0:K])
            wstep = const.tile([1, K], F32, name="wstep")
            nc.vector.tensor_mul(wstep, wdiff, gemask)
            wc = const.tile([1, K], F32, name="wc")
            nc.vector.tensor_add(wc, w_sb[:, 0:K], wstep)
            wc64 = const.tile([1, K], F32, name="wc64")
            nc.vector.tensor_scalar(out=wc64, in0=wc, scalar1=1.0 / W2SCALE,
                                    scalar2=None, op0=mybir.AluOpType.mult)
            nc.gpsimd.partition_broadcast(wbc3[:, 0, :], wc64, channels=P)
            zb2sum = const.tile([1, D], F32, name="zb2sum")
            nc.vector.tensor_scalar(out=zb2sum, in0=zb2_sb[:, 0, :],
                                    scalar1=w_sb[:, 0:1], scalar2=None,
                                    op0=mybir.AluOpType.mult)
            for e in range(1, E):
                nc.vector.scalar_tensor_tensor(out=zb2sum, in0=zb2_sb[:, e, :],
                                               scalar=w_sb[:, e:e + 1], in1=zb2sum,
                                               op0=mybir.AluOpType.mult,
                                               op1=mybir.AluOpType.add)
            zb2b3 = const.tile([P, 1, D], F32, name="zb2b3")
            nc.gpsimd.partition_broadcast(zb2b3[:, 0, :], zb2sum, channels=P)
            for t in range(TC):
                nc.vector.scalar_tensor_tensor(out=zacc[t], in0=x_all[:, t, :],
                                               scalar=1.0, in1=zb2b3[:, 0, :],
                                               op0=mybir.AluOpType.mult,
                                               op1=mybir.AluOpType.add)

            # preload the GELU activation table after the softmax Exp (the
            # w_sb bias forces exp -> gelu order on the scalar stream)
            gelu_warm = const.tile([1, DC], F32, name="gelu_warm")
            nc.scalar.activation(out=gelu_warm, in_=pooled_f[0:1, :], func=GELU,
                                 bias=w_sb[:, 0:1])

            # ---------- main loop ----------
            NPAIR = FC // 2
            for k in range(K):
                for fc in range(FC):
                    if fc % 2 == 0:
                        nfcp = fc // 2 + PRE1
                        if nfcp < NPAIR:
                            load_zw1(k, nfcp)
                        elif k + 1 < K:
                            load_zw1(k + 1, nfcp - NPAIR)
                    nxt28 = zw28_sched.get((k, fc))
                    if nxt28 is not None:
                        load_zw28(*nxt28)
                    p1 = ps.tile([P, S], F32, name=f"p1_{k}_{fc}", tag="p1", bufs=3)
                    w1t = zw1q[(k, fc // 2)]
                    for dcp in range(DP):
                        nc.tensor.matmul(p1, w1t[:, fc % 2, dcp, :, :], xT8[dcp],
                                         start=(dcp == 0), stop=(dcp == DP - 1),
                                         perf_mode=DR)
                    nc.scalar.activation(out=hid8[k % 2][:, fc, :], in_=p1,
                                         func=GELU, bias=zb1_sb[k][:, fc:fc + 1],
                                         scale=1.0 / W1SCALE)
                    if k > 0 or fc >= 16:
                        drain_g2(1)
            # tail: drain remaining G2 work (expert 2 fch=1 groups)
            drain_g2(16)

    nc.finalize()
    return nc


_NC_CACHE = None


def _get_nc():
    global _NC_CACHE
    if _NC_CACHE is None:
        _NC_CACHE = build_nc()
    return _NC_CACHE


def kernel(x, rw1, rb1, rw2, rb2, zw1, zb1, zw2, zb2, **run_kwargs):
    x = np.asarray(x, dtype=np.float32)
    zw1 = np.asarray(zw1, np.float32)
    zw2 = np.asarray(zw2, np.float32)
    zb1 = np.asarray(zb1, np.float32)
    # zw1r8[e*P+p, fcp, j, dcp, i, fw] = 32*zw1[e, (2*dcp+i)*P+p, (2*fcp+j)*P+fw]
    zw1r = np.ascontiguousarray(
        (zw1 * W1SCALE).reshape(E, DP, 2, P, FC // 2, 2, P)
        .transpose(0, 3, 4, 5, 1, 2, 6)
        .reshape(E * P, FC // 2, 2, DP, 2, P)).astype(ml_dtypes.float8_e4m3)
    zb1r = np.ascontiguousarray(
        zb1.reshape(E, FC, P).transpose(0, 2, 1).reshape(E * P, FC))
    # zw28[(e*2+dh)*P+p, fcp, i, j] = 64*zw2[e, (2*fcp+i)*P+p, dh*512+j]
    zw28 = np.ascontiguousarray(
        (zw2 * W2SCALE).reshape(E, FC // 2, 2, P, 2, 512)
        .transpose(0, 4, 3, 1, 2, 5)
        .reshape(E * 2 * P, FC // 2, 2, 512)).astype(ml_dtypes.float8_e4m3)
    shared = {
        "rw1b": np.asarray(rw1, np.float32).astype(ml_dtypes.bfloat16),
        "rb1": np.ascontiguousarray(np.asarray(rb1, np.float32)),
        "rw2": np.ascontiguousarray(np.asarray(rw2, np.float32)),
        "rb2": np.ascontiguousarray(np.asarray(rb2, np.float32)),
        "zw1r": zw1r,
        "zb1r": zb1r,
        "zw28": zw28,
        "zb2": np.ascontiguousarray(np.asarray(zb2, np.float32)),
    }
    B = x.shape[0]
    nc = _get_nc()
    in_maps = []
    for b in range(B):
        xb_ = x[b]
        xt = np.ascontiguousarray(xb_.T)            # [D, S] fp32
        m = dict(shared, x=xb_.astype(ml_dtypes.bfloat16),
                 xt8=np.ascontiguousarray(
                     xt.reshape(DP, 2, P, S).transpose(0, 2, 1, 3)
                     .reshape(DP * P, 2, S)).astype(ml_dtypes.float8_e4m3))
        in_maps.append(m)
    res = run_bass_kernel_spmd(nc, in_maps, core_ids=list(range(B)), **run_kwargs)
    out = np.stack([res.results[b]["out"] for b in range(B)], axis=0)
    if run_kwargs:
        kernel.last_results = res
    return out


if __name__ == "__main__":
    rng = np.random.default_rng(0)
    inputs = {
        "x": rng.standard_normal((8, S, D)).astype(np.float32),
        "rw1": (rng.standard_normal((D, H)) / np.sqrt(D)).astype(np.float32),
        "rb1": np.zeros(H, np.float32),
        "rw2": (rng.standard_normal((H, E)) / np.sqrt(H)).astype(np.float32),
        "rb2": np.zeros(E, np.float32),
        "zw1": (rng.standard_normal((E, D, F)) / np.sqrt(D)).astype(np.float32),
        "zb1": np.zeros((E, F), np.float32),
        "zw2": (rng.standard_normal((E, F, D)) / np.sqrt(F)).astype(np.float32),
        "zb2": np.zeros((E, D), np.float32),
    }
    out = kernel(**inputs)
    print("out", out.shape, out.dtype, np.abs(out).max())
